# revision 40
# baseline (speedup 1.0000x reference)
"""Bbox regression loss (smooth-L1 over gathered bbox deltas) on 8 TRN2 cores.

The loss gathers 4 scalars per (batch, gt-box) from each FPN level's dense
prediction tensor, applies smooth-L1 against the gt deltas, and reduces to
two scalars (weighted loss sum, valid-box count).  Only 3 x 2 x 128 x 4 =
3072 elements of the ~92MB of predictions are ever read, so the kernel is
built around one on-device dma_gather rather than streaming.

Sharding: core c handles (b = c//4, k = c%4) where k indexes the 4 bbox
coordinate channels (channel group k*A:(k+1)*A of the 4*A=12 channel dim).
Each core receives exactly 1/8 of every prediction tensor (concatenated
into one row table), computes per-gt-entry partials fully on device, and
the host sums the 8 x 128 partial rows (the sharding hint's all-reduce).

Critical path per core = 3 chained DMAs; everything else is hidden:
  1. one 64B/partition HWDGE load (auxi) carrying the packed int16 gather
     row indices + the output scatter indices, all precomputed on host
     from the (small) coord tensors.  A second HWDGE load (aux) with gt /
     validity / in-row offsets lands under the gather's shadow.  Masked
     entries (pad gt or inactive sample) point at a zero pad row appended
     to the table with gt=0, so they contribute exactly 0 loss with no
     on-device masking.
  2. main dma_gather, PREPARE_ONLY + trigger_dma (saves the 650ns
     DGE-delay of the normal SWDGE path): 384 rows of 512B from the
     concatenated prediction table -> g[m, level, 128] f32.
  3. DVE-only math: fused (iota==rem)*g one-hot select with per-partition
     accumulate -> pred[m,l] (iota built by a prefix-scan; TensorScalarPtr
     ops are DVE-only -- walrus rejects them on Pool), then smooth-L1 in 4
     ops via the Huber factorization 2*sl(d) = c*(2d-c), c = clamp(d,-1,1)
     (0.5 folded into the host-side weight), written next to the validity
     columns.
  4. output via a PREPARE_ONLY dma_scatter_add of 128 DISTINCT 64B rows
     (idx = iota shipped in auxi; the runtime collapses duplicate scatter
     indices, so the reduction is done by the host over the 128 rows).
     The prep runs in the gather window; the trigger fires right after the
     last vector op.

Tile/runtime impedance fixes baked in after TileContext exit: consumer
waits are repointed from Tile's never-bumped DMASW lane sems to the preps'
own descriptor-baked completion sems (the only wiring that orders
consumers after a triggered DMA on both the runtime and the cost model),
the end-of-program drain's wait on the output scatter is dropped (the
runtime drains DMA rings before returning outputs), and the custom sems
are cleared at program start so repeat runs see fresh counts.

Template fat removed (each verified on the runtime, including cold-start
and sequential different-input runs): the four const-AP warm-up memsets
Bass.__init__ emits on Pool (nothing here reads those consts), the
all-engine start barrier (every cross-engine ordering in this program is
semaphore-wired; removing it symmetrically is safe, unlike skipping one
engine's waits), and both epilogue barriers around the semaphore clears
(the drain's global-clock waits already cover every engine's last tick
and the DMA-ring quiesce).
"""

import os

import numpy as np

try:  # persistent XLA/NEFF compile cache across processes
    import jax

    os.makedirs("/tmp/jax_pcache", exist_ok=True)
    jax.config.update("jax_compilation_cache_dir", "/tmp/jax_pcache")
    jax.config.update("jax_persistent_cache_min_compile_time_secs", 0.0)
    jax.config.update("jax_persistent_cache_min_entry_size_bytes", 0)
except Exception:
    pass

import concourse.bacc as bacc
import concourse.bass as bass
import concourse.tile as tile
from concourse import mybir
from concourse.bass_utils import run_bass_kernel_spmd

A = 3                       # anchors per level
M = 128                     # gt entries per sample
GRIDS = (96, 48, 24)        # level l grid; level l uses coord/diff index 2-l
LOSS_W = (1.0, 1.0, 1.0, 0.1)
ROW = 128                   # f32 elements per gather row (512B)
NLVL = 3
NIDX = NLVL * M             # 384 gathered rows per core
V = tuple(A * g * g * g // ROW for g in GRIDS)      # (20736, 2592, 324)
VBASE = (0, V[0], V[0] + V[1])
VTOT = sum(V)               # 23652 rows; +1 zero pad row < int16 max
N_CORES = 8

AUXC = 64                   # aux row: 256B gather granularity
# aux f32 columns: 0:3 gt | 3:6 validf | 6:9 remf | 10:22 idx16 (bitcast)
IDXC = 10

F32 = mybir.dt.float32
I16 = mybir.dt.int16
Alu = mybir.AluOpType


def _build_bass() -> bass.Bass:
    # Bass.__init__ unconditionally memsets four const-AP scratch tensors on
    # Pool before the program's start barrier; nothing in this kernel reads
    # them, yet Pool's ~400ns of warm-up gates the all-engine barrier release
    # and therefore the first input DMA.  Suppress the emission (the tensors
    # are still allocated; the barrier itself is kept).
    _orig_memset = bass.BassEitherVectorEngine.memset
    _orig_barrier = bacc.Bacc.all_engine_barrier
    bass.BassEitherVectorEngine.memset = lambda self, ap, c: None
    bacc.Bacc.all_engine_barrier = lambda self: None
    try:
        nc = bacc.Bacc(
            "TRN2",
            target_bir_lowering=False,
            debug=False,
            num_devices=N_CORES,
            num_swdge_queues=2,
        )
    finally:
        bass.BassEitherVectorEngine.memset = _orig_memset
        bacc.Bacc.all_engine_barrier = _orig_barrier
    tab = nc.dram_tensor("tab", [VTOT + 1, ROW], F32, kind="ExternalInput")
    auxi = nc.dram_tensor("auxi", [M, 16], F32, kind="ExternalInput")
    auxd = nc.dram_tensor("aux", [M, AUXC], F32, kind="ExternalInput")
    out = nc.dram_tensor("partial", [M, AUXC], F32, kind="ExternalOutput")

    s_g = nc.alloc_semaphore("g_dma")
    s_out = nc.alloc_semaphore("out_dma")

    # Tile's epilogue is drain -> barrier -> sem-clears -> barrier; the final
    # barrier only keeps the other (long-idle) engines from halting while
    # Pool finishes the clears, which the NEFF's own completion already
    # guarantees.  Skip it (~230ns off the program end).
    from concourse.vector_clock import ScopedClock as _SC

    def _drain_no_final_barrier(self, tick_clock, wait_clock):
        # the drain's global-clock waits already cover every engine's last
        # tick (and the DMA ring quiesce), so the pre-clear barrier is
        # redundant for this program too
        drain_inst = self.nc.sync.drain()
        wait_clock.add_sem_waits(
            drain_inst.ins, _SC({None: tick_clock.global_clock})
        )
        assert self.sems is not None
        popped = self.nc._tile_sem_poison_stack.pop()
        assert popped is self._sem_poison
        self.nc.clear_and_free_semaphores(list(self.sems.allocated().values()))

    _orig_drain = tile.TileContext._drain_and_barrier
    tile.TileContext._drain_and_barrier = _drain_no_final_barrier
    try:
        _tc_ctx = tile.TileContext(nc)
    finally:
        tile.TileContext._drain_and_barrier = _orig_drain
    _tc_ctx_patched = _tc_ctx
    _tc_ctx_patched._drain_and_barrier = _drain_no_final_barrier.__get__(_tc_ctx_patched)
    with _tc_ctx_patched as tc:
        with tc.tile_pool(name="sb", bufs=1) as sb:
            # the custom DMA-completion sems are outside Tile's epilogue
            # clear; zero them at program start or the second run's >=16
            # waits would be pre-satisfied by the first run's bumps
            nc.gpsimd.sem_clear(s_g)
            nc.gpsimd.sem_clear(s_out)
            aux = sb.tile([M, AUXC], F32)
            auxit = sb.tile([M, 16], F32)
            g = sb.tile([M, NLVL, ROW], F32)
            io = sb.tile([M, ROW], F32)
            ones = sb.tile([M, ROW], F32)
            pred = sb.tile([M, NLVL], F32)
            scr0 = sb.tile([M, ROW], F32)
            scr1 = sb.tile([M, ROW], F32)
            scr2 = sb.tile([M, ROW], F32)
            d = sb.tile([M, NLVL], F32)
            t1 = sb.tile([M, NLVL], F32)
            q = sb.tile([M, NLVL], F32)

            # --- aux loads via HWDGE from SP: for the head-of-program DMA
            # (no waits) SEQ+HWDGE gen overlap the start barrier, beating a
            # SWDGE prep+trigger.  The 48B idx payload goes first/alone so
            # the gather prep can start ~125ns earlier. ---
            nc.sync.dma_start(out=auxit[:], in_=auxi[:])
            nc.sync.dma_start(out=aux[:], in_=auxd[:])

            # the extraction iota on the (otherwise idle) DVE, via a
            # prefix scan of ones (InstIota is Pool-only and Pool pre-work
            # would delay the gather prep in its in-order stream)
            nc.vector.memset(ones[:], 1.0)
            nc.vector.tensor_tensor_scan(
                io[:], ones[:], ones[:], -1.0, Alu.add, Alu.bypass
            )

            # --- main gather: 384 rows of 512B; prep waits only on auxi ---
            idx16 = auxit[:, 0 : NIDX // 16 // 2].bitcast(I16)
            nc.gpsimd.dma_gather(
                g[:], tab[:], idx16, NIDX, NIDX, ROW,
                prepare_only=True, queue_num=0, sem=s_g,
            )
            nc.gpsimd.trigger_dma(count=None, queue_num=0)

            # --- output scatter-add: 128 DISTINCT rows (idx = iota shipped
            # in auxi cols 12:16), one token per gt entry; the host sums the
            # 128 partials.  Distinct rows sidestep the runtime collapsing
            # duplicate scatter indices, and the whole reduction tail
            # (matmul + PSUM copy + their sem hops) disappears. ---
            zi = auxit[:, 12:16].bitcast(I16)
            aux3 = aux[:, 0:16].rearrange("p (a f) -> p a f", a=1)
            nc.gpsimd.dma_scatter_add(
                out[:, 0:16], aux3, zi, M, M, 16, elem_step=AUXC,
                prepare_only=True, queue_num=1, sem=s_out,
            )

            # pred[m,l] = g[m,l,rem[m,l]] -- fused (iota==rem)*g + row-sum.
            # All on DVE: per-partition-scalar ops (TensorScalarPtr) only
            # exist there (walrus rejects them on Pool).
            gts = aux[:, 0:3]
            remf = aux[:, 6:9]
            for lvl, scr in ((0, scr0), (1, scr1), (2, scr2)):
                nc.vector.scalar_tensor_tensor(
                    out=scr[:],
                    in0=io[:],
                    scalar=remf[:, lvl : lvl + 1],
                    in1=g[:, lvl, :],
                    op0=Alu.is_equal,
                    op1=Alu.mult,
                    accum_out=pred[:, lvl : lvl + 1],
                )

            # smooth l1 (x2) in 4 ops via the Huber factorization
            #   2*sl(d) = c * (2d - c),  c = clamp(d, -1, 1)
            # (|d|<1: c=d -> d*d; d>=1: 2d-1; d<=-1: -(2d+1) = 2|d|-1;
            #  the 0.5 lives in the host-side weight)
            nc.vector.tensor_tensor(d[:], pred[:], gts, Alu.subtract)
            nc.vector.tensor_scalar(t1[:], d[:], -1.0, 1.0, Alu.max, Alu.min)
            nc.vector.scalar_tensor_tensor(   # 2d - c
                out=q[:], in0=d[:], scalar=2.0, in1=t1[:],
                op0=Alu.mult, op1=Alu.subtract,
            )
            # sl2 lands in aux[:,0:3], next to validf in 3:6; junk in the
            # remaining scattered columns is summed into out cols the host
            # never reads.
            nc.vector.tensor_tensor(aux[:, 0:3], t1[:], q[:], Alu.mult)
            nc.gpsimd.trigger_dma(count=None, queue_num=1)

    # Tile points every consumer wait (and the end-of-program drain) at
    # its per-lane DMASW tick semaphores, but for PREPARE_ONLY preps nothing
    # ever bumps those lanes: the DMA completion bump goes to the user sem=
    # baked into the descriptor (on_update[0]).  Redirect the waits to the
    # prep's own sem -- empirically the only wiring that orders consumers
    # after the triggered DMA on both the runtime and the cost model.
    from concourse.tile_scheduler import PROC_NAMES

    fn = nc.m.functions[0]
    lane_to_sem: dict[str, tuple[int, str]] = {}
    out_lanes: set[str] = set()
    for bb in fn.blocks:
        for ins in bb.instructions:
            if getattr(ins, "gen_mode", 0) != 1:
                continue
            lane = PROC_NAMES[ins.bass_scheduled_proc]
            assert lane.startswith("DMASW"), lane
            u0 = ins.sync_info.on_update[0]
            lane_to_sem[lane] = (u0.id, u0.ant_name)
            if isinstance(ins, mybir.InstDMAScatterAddAnt):
                out_lanes.add(lane)
    for bb in fn.blocks:
        for ins in bb.instructions:
            si = ins.sync_info
            if si is None:
                continue
            for w in si.on_wait:
                lane = w.ant_name.split("_")[0] if w.ant_name else ""
                if lane not in lane_to_sem:
                    continue
                if lane in out_lanes and type(ins).__name__ == "InstDrain":
                    # nothing on-device consumes the output scatter; the
                    # runtime drains DMA rings before returning outputs, so
                    # the end-of-program drain need not serialize on it
                    w.wait_value = 0
                else:
                    w.id, w.ant_name = lane_to_sem[lane]

    nc.finalize()
    return nc


_sidx = np.zeros((16, 8), np.int16)
for _i in range(M):
    _sidx[_i % 16, _i // 16] = _i
_SCATTER_IDX = np.tile(_sidx, (8, 1)).view(np.float32)

_NC = None


def _get_nc():
    global _NC
    if _NC is None:
        _NC = _build_bass()
    return _NC


def kernel(**inputs: np.ndarray):
    out_l = [np.asarray(inputs[n]) for n in ("out1", "out3", "out5")]
    # level l uses coord/diff (2-l)  (the reference pairs them reversed)
    coords = [np.asarray(inputs[f"coord{2 - l}"]) for l in range(3)]
    diffs = [np.asarray(inputs[f"diff{2 - l}"]) for l in range(3)]

    in_maps = []
    for c in range(N_CORES):
        b, k = c // 4, c % 4
        im = {}
        im["tab"] = np.concatenate(
            [
                np.ascontiguousarray(out_l[l][b, A * k : A * (k + 1)]).reshape(
                    V[l], ROW
                )
                for l in range(3)
            ]
            + [np.zeros((1, ROW), np.float32)],
            axis=0,
        )
        aux = np.zeros((M, AUXC), np.float32)
        rows = np.zeros((M, NLVL), np.int64)
        for l, g in enumerate(GRIDS):
            cc = coords[l][b].astype(np.int64)  # [128, 4]
            valid = cc[:, 0] > -1
            active = bool(cc[0, 0] > -1)
            mask = valid & active
            a = np.maximum(cc[:, 0], 0)
            flat = ((a * g + cc[:, 1]) * g + cc[:, 2]) * g + cc[:, 3]
            row = VBASE[l] + (flat >> 7)
            rem = flat & (ROW - 1)
            row[~mask] = VTOT          # zero pad row
            rem[~mask] = 0
            rows[:, l] = row
            aux[:, 6 + l] = rem.astype(np.float32)
            aux[:, 3 + l] = mask.astype(np.float32)
            aux[mask, l] = diffs[l][b, mask, k]
        # wrapped idx layout: item i = l*128 + m -> idxw[i%16, i//16],
        # replicated across the 8 gpsimd cores (partition p reads p%16 row)
        idxw = np.zeros((16, NIDX // 16), np.int16)
        for l in range(NLVL):
            for mm in range(M):
                i = l * M + mm
                idxw[i % 16, i // 16] = rows[mm, l]
        auxi = np.zeros((M, 16), np.float32)
        auxi[:, 0 : NIDX // 16 // 2] = np.tile(idxw, (8, 1)).view(np.float32)
        auxi[:, 12:16] = _SCATTER_IDX
        im["auxi"] = auxi
        im["aux"] = aux
        in_maps.append(im)

    res = run_bass_kernel_spmd(_get_nc(), in_maps, core_ids=list(range(N_CORES)))
    # host epilogue of the reduction: per-core constant loss-weight scaling
    # (0.5*LOSS_W[k], weight counted once via the k==0 cores) + all-reduce
    loss = np.float32(0.0)
    weight = np.float32(0.0)
    for c in range(N_CORES):
        k = c % 4
        p = res.results[c]["partial"]
        loss += np.float32(p[:, 0:3].sum() * np.float32(0.5 * LOSS_W[k]))
        if k == 0:
            weight += np.float32(p[:, 3:6].sum())
    return (np.array([loss], np.float32), np.array([weight], np.float32))
